# revision 10
# baseline (speedup 1.0000x reference)
"""PianoSSM Trainium2 kernel (nn_PianoSSM_XL).

Model: 4 stacked diagonal-complex SSM layers + linear head over
L = 512*32 = 16384 timesteps, batch 8. Data-parallel: one batch element
per NeuronCore (8 cores).

Math decomposition (per layer, lam_bar = r * e^{i theta} per channel):
  true state X[t] = sum_{s<=t} lam^{t-s} Bu[s]
  split t = 32*b + j.  Within a block, work in the rotated frame
  Z[b,j] = sum_{s<=j} r^{j-s} (e^{-i th s} Bu[32b+s])   (block-LOCAL, resets)
  - e^{-i th j} folded into 32 B-side weight variants (Btil_j)
  - Z computed by hardware tensor_tensor_scan (2 real scans per channel
    group; data0 pattern = r with 0 at block starts -> per-block reset)
  - block-local finals F_b = e^{i th 31} Z[b,31]; carries
    X_b = lam^32 X_{b-1} + F_b via a block-rate scan (rotation trick)
  - y[32b+j] = Re(C2_j Z[b,j]) + Re(C3_j W_b) + skip + bias, where
    W_b = e^{i th} X_{b-1},  C2_j = sqrt2 * C o e^{i th j},
    C3_j = C2_j o r^{j+1}    (relu prescale 1/sqrt2 folded into C-side)
  Layer 0 input is block-constant (midi repeated 32x): no full-rate scan;
  y = Re(C1_j X_{b-1}) + Re(C2S_j Bu_mid[b]) with C1_j = sqrt2*C o lam^{j+1},
  C2S_j = sqrt2*C o S_{j+1}, S_k = (lam^k - 1)/(lam - 1).

Layouts: everything "transposed" (features on partitions, time on free dim).
u buffers are j-major per chunk (col = j*NBc + b) so every per-j matmul
reads/writes contiguous stripes. v / Z are t-order (col = 32*b + j) for the
scans; per-j access is stride-32.
"""
import numpy as np

# ---------------- model constants (hardcoded per problem spec) -------------
T = 32                 # block size == COPY
LAYERS = [(88, 256, 88, False), (88, 256, 60, True), (60, 256, 40, True),
          (40, 256, 20, True)]
SQRT2 = float(np.sqrt(2.0))
N_CORES = 8
P_FULL = 256           # state size (2 groups of 128)
G = 2                  # channel groups


# ---------------- host-side weight preprocessing (float64) -----------------
def _prep_layer(ssm):
    dt = np.exp(np.asarray(ssm['log_step'], np.float64))
    lam = np.asarray(ssm['lam_re'], np.float64) + 1j * np.asarray(ssm['lam_im'], np.float64)
    lam_bar = np.exp(lam * dt)
    r = np.abs(lam_bar)
    th = np.angle(lam_bar)
    B = np.asarray(ssm['B_re'], np.float64) + 1j * np.asarray(ssm['B_im'], np.float64)
    B_bar = ((lam_bar - 1.0) / lam)[:, None] * B            # [P,H]
    C = np.asarray(ssm['C_re'], np.float64) + 1j * np.asarray(ssm['C_im'], np.float64)
    return lam_bar, r, th, B_bar, C


def _pack_cfam(Cfam):
    """Cfam: [T, O, P] complex -> dram [T, 128, 4*O] fp16 with im NEGATED.
    col layout: (g*2 + comp)*O + o ; partition k = K-row within group."""
    Tv, O, P = Cfam.shape
    out = np.zeros((Tv, 128, 4 * O), np.float16)
    for g in range(G):
        blk = Cfam[:, :, g * 128:(g + 1) * 128]             # [T,O,128]
        out[:, :, (g * 2 + 0) * O:(g * 2 + 1) * O] = \
            blk.real.transpose(0, 2, 1).astype(np.float16)
        out[:, :, (g * 2 + 1) * O:(g * 2 + 2) * O] = \
            (-blk.imag).transpose(0, 2, 1).astype(np.float16)
    return out


def _pack_bfam(Bfam):
    """Bfam: [T, P, H] complex -> dram [T, H, 512] fp16 (lhsT per comp/group).
    col = comp*256 + g*128 + m."""
    Tv, P, H = Bfam.shape
    out = np.zeros((Tv, H, 512), np.float16)
    for comp, arr in enumerate((Bfam.real, Bfam.imag)):
        for g in range(G):
            out[:, :, comp * 256 + g * 128: comp * 256 + (g + 1) * 128] = \
                arr[:, g * 128:(g + 1) * 128, :].transpose(0, 2, 1).astype(np.float16)
    return out


def _tab_pack(tab):
    """tab [P_FULL, N] complex -> [2, 2, 128, N] f32 (comp, g)."""
    N = tab.shape[1]
    out = np.zeros((2, G, 128, N), np.float32)
    for g in range(G):
        out[0, g] = tab[g * 128:(g + 1) * 128, :].real.astype(np.float32)
        out[1, g] = tab[g * 128:(g + 1) * 128, :].imag.astype(np.float32)
    return out


def preprocess(midi, params, L):
    """Returns (shared device input dict, per-core midiT list)."""
    NBLK = L // T
    MIDI = NBLK                       # midi steps == blocks (COPY == T)
    j = np.arange(T)
    b = np.arange(NBLK)
    ins = {}

    for li, (H, P, O, skip) in enumerate(LAYERS):
        lp = params['L%d' % li]
        lam, r, th, B_bar, C = _prep_layer(lp['ssm'])
        b_in = np.asarray(lp['ssm']['b_in'], np.float64)
        b_out = np.asarray(lp['ssm']['b_out'], np.float64)
        thT = th * T
        rT = r ** T
        sC = (2.0 / SQRT2) * C        # sqrt2 * C  (relu prescale folded)

        if li == 0:
            # --- all-matmul layer ---
            denom = np.where(np.abs(lam - 1.0) < 1e-30, 1.0, lam - 1.0)
            S = (lam[None, :] ** (j + 1)[:, None] - 1.0) / denom[None, :]  # S_{j+1} [T,P]
            C1 = sC[None] * (lam[None, :] ** (j + 1)[:, None])[:, None, :]   # [T,O,P]
            C2S = sC[None] * S[:, None, :]
            ins['L0_c1'] = _pack_cfam(C1)
            ins['L0_c2s'] = _pack_cfam(C2S)
            # B lhsT [88, 512] (col = comp*256 + m)
            bl = np.zeros((H, 512), np.float16)
            bl[:, 0:256] = B_bar.real.T.astype(np.float16)
            bl[:, 256:512] = B_bar.imag.T.astype(np.float16)
            ins['L0_b'] = bl
            ST = S[T - 1, :]          # sum_{i<T} lam^i
            t0pre = np.exp(-1j * thT[None, :] * b[:, None]).T * ST[:, None]   # [P,NBLK]
            ins['L0_t0pre'] = _tab_pack(t0pre)
            # t0post col c (global) multiplies Xtil_{c-1}: e^{+i thT (c-1)}
            t0post = np.exp(1j * thT[:, None] * (b - 1)[None, :])             # [P,NBLK]
            ins['L0_t0post'] = _tab_pack(t0post)
            ins['L0_r32'] = _tab_pack(rT[:, None].astype(np.complex128))[0]   # [2,128,1]
            bbin = B_bar @ b_in                                               # [P] complex
            ins['L0_bbin'] = _tab_pack(bbin[:, None])                         # [2,2,128,1]
            ins['L0_bout'] = (b_out / SQRT2).astype(np.float32)[:, None]
        else:
            rot_in = np.exp(-1j * th[None, :] * j[:, None])                   # [T,P]
            Btil = rot_in[:, :, None] * B_bar[None]                           # [T,P,H]
            ins['L%d_wb' % li] = _pack_bfam(Btil)
            C2 = sC[None] * np.exp(1j * th[None, :] * j[:, None])[:, None, :]  # [T,O,P]
            C3 = C2 * (r[None, :] ** (j + 1)[:, None])[:, None, :]
            ins['L%d_wc2' % li] = _pack_cfam(C2)
            ins['L%d_wc3' % li] = _pack_cfam(C3)
            ins['L%d_skip' % li] = np.asarray(lp['skip_W'], np.float64).astype(np.float16)
            # tabF: F_b = e^{i th (T-1)} Z_end; pre-rot for block scan e^{-i thT b}
            tabF = np.exp(1j * th[:, None] * (T - 1) - 1j * thT[:, None] * b[None, :])
            ins['L%d_tabF' % li] = _tab_pack(tabF)
            # tabW col b multiplies Xtil_{b-1}: W_b = e^{i th (T b - (T-1))} Xtil_{b-1}
            tabW = np.exp(1j * th[:, None] * (T * b[None, :] - (T - 1)))
            ins['L%d_tabW' % li] = _tab_pack(tabW)
            ins['L%d_r32' % li] = _tab_pack(rT[:, None].astype(np.complex128))[0]
            ins['L%d_r' % li] = _tab_pack(r[:, None].astype(np.complex128))[0]
            # b_in fold: cv[comp, g, :, j] = (Btil_j @ b_in)
            cv = np.einsum('tph,h->tp', Btil, b_in)                           # [T,P]
            cvp = np.zeros((2, G, 128, T), np.float32)
            for g in range(G):
                cvp[0, g] = cv.T[g * 128:(g + 1) * 128, :].real.astype(np.float32)
                cvp[1, g] = cv.T[g * 128:(g + 1) * 128, :].imag.astype(np.float32)
            ins['L%d_cv' % li] = cvp
            ins['L%d_bout' % li] = (b_out / SQRT2 +
                                    np.asarray(lp['skip_b'], np.float64)
                                    ).astype(np.float32)[:, None]
            # NOTE: skip_b folded into bout bias (added before relu would be
            # wrong -> handled below: bias arg of relu only gets b_out/sqrt2;
            # skip_b added via skip PSUM? We fold skip_b into the DVE add via
            # bias-free trick: skip_b is added to bout AFTER relu is wrong.
            # -> corrected in kernel: relu bias = b_out/sqrt2 only; skip_b
            # separate [O,1] added with the skip term.
            ins['L%d_bout' % li] = (b_out / SQRT2).astype(np.float32)[:, None]
            ins['L%d_skipb' % li] = np.asarray(lp['skip_b'], np.float64
                                               ).astype(np.float32)[:, None]

    ins['headW'] = np.asarray(params['head_W'], np.float64).astype(np.float16)  # [20,1]
    ins['headb'] = np.asarray(params['head_b'], np.float64).astype(np.float32)[:, None]  # [1,1]

    midiT = [np.ascontiguousarray(np.asarray(midi[bi], np.float32).T
                                  ).astype(np.float16) for bi in range(midi.shape[0])]
    return ins, midiT


# ---------------- device kernel ------------------------------------------
def build_nc(L, n_chunks=2):
    import concourse.bass as bass
    import concourse.bacc as bacc
    import concourse.tile as tile
    from concourse import mybir

    F32 = mybir.dt.float32
    F16 = mybir.dt.float16
    AF = mybir.ActivationFunctionType
    OP = mybir.AluOpType

    NBLK = L // T
    MIDI = NBLK
    CT = L // n_chunks            # timesteps per chunk
    NBc = CT // T                 # blocks per chunk
    SEG_BLKS = min(32, NBc)
    SEG = SEG_BLKS * T            # scan segment cols
    NSEG = CT // SEG

    nc = bacc.Bacc("TRN2", target_bir_lowering=False, debug=False)

    # ---- DRAM I/O ----
    dr = {}
    def din(name, shape, dt):
        dr[name] = nc.dram_tensor(name, list(shape), dt, kind="ExternalInput").ap()

    din('midiT', (88, MIDI), F16)
    din('L0_b', (88, 512), F16)
    din('L0_c1', (T, 128, 4 * 88), F16)
    din('L0_c2s', (T, 128, 4 * 88), F16)
    din('L0_t0pre', (2, G, 128, MIDI), F32)
    din('L0_t0post', (2, G, 128, MIDI), F32)
    din('L0_r32', (2, 128, 1), F32)
    din('L0_bbin', (2, G, 128, 1), F32)
    din('L0_bout', (88, 1), F32)
    for li in (1, 2, 3):
        H, P, O, _ = LAYERS[li]
        din('L%d_wb' % li, (T, H, 512), F16)
        din('L%d_wc2' % li, (T, 128, 4 * O), F16)
        din('L%d_wc3' % li, (T, 128, 4 * O), F16)
        din('L%d_skip' % li, (H, O), F16)
        din('L%d_tabF' % li, (2, G, 128, NBLK), F32)
        din('L%d_tabW' % li, (2, G, 128, NBLK), F32)
        din('L%d_r32' % li, (2, 128, 1), F32)
        din('L%d_r' % li, (2, 128, 1), F32)
        din('L%d_cv' % li, (2, G, 128, T), F32)
        din('L%d_bout' % li, (O, 1), F32)
        din('L%d_skipb' % li, (O, 1), F32)
    din('headW', (20, 1), F16)
    din('headb', (1, 1), F32)
    audio_out = nc.dram_tensor("audio", [1, L], F32, kind="ExternalOutput").ap()

    with tile.TileContext(nc) as tc:
        with tc.tile_pool(name="singles", bufs=1) as singles, \
             tc.tile_pool(name="wb_pool", bufs=4) as wb_pool, \
             tc.tile_pool(name="wc_pool", bufs=6) as wc_pool, \
             tc.tile_pool(name="tab_pool", bufs=6) as tab_pool, \
             tc.tile_pool(name="tmp_pool", bufs=2) as tmp_pool, \
             tc.tile_pool(name="pat_pool", bufs=2) as pat_pool, \
             tc.tile_pool(name="mm_ps", bufs=4, space="PSUM") as mm_ps, \
             tc.tile_pool(name="y_ps", bufs=2, space="PSUM") as y_ps_pool, \
             tc.tile_pool(name="s_ps", bufs=2, space="PSUM") as s_ps_pool:

            # ---------- persistent SBUF ----------
            # u ping-pong (j-major per chunk) fp16
            u_a = singles.tile([88, CT], F16, tag="u_a")
            u_b = singles.tile([88, CT], F16, tag="u_b")
            # v (t-order, scan input) fp32 — shared serially across groups
            v_buf = singles.tile([128, CT], F16, tag="v_buf")
            # Z per group fp16
            Z = {(g, c): singles.tile([128, CT], F16, tag=f"Z_{g}_{c}", name=f"Z_{g}_{c}")
                 for g in range(G) for c in range(2)}
            # scan reset patterns per (layer 1..3, group) [128, SEG]
            pat = {}
            # block-scan data0 (r^T) per (layer, group): [128, NBc]
            r32m = {}
            # Xtil carry buffers per (layer, comp, g): [128, NBc+1]
            xtil = {(li, c, g): singles.tile([128, NBc + 1], F16,
                                             tag=f"xt_{li}_{c}_{g}",
                                             name=f"xt_{li}_{c}_{g}")
                    for li in range(4) for c in range(2) for g in range(G)}
            audio_sb = singles.tile([T, NBLK], F32, tag="audio")

            # small per-layer constants kept resident
            smalls = {}
            def load_small(name, shape, dt=F32):
                t = singles.tile(list(shape), dt, tag=name)
                nc.sync.dma_start(out=t, in_=dr[name])
                smalls[name] = t
                return t

            midiT_sb = singles.tile([88, MIDI], F16, tag="midiT")
            nc.sync.dma_start(out=midiT_sb, in_=dr['midiT'])
            l0b_sb = singles.tile([88, 512], F16, tag="l0b")
            nc.sync.dma_start(out=l0b_sb, in_=dr['L0_b'])
            load_small('L0_bout', (88, 1))
            load_small('headW', (20, 1), F16)
            load_small('headb', (1, 1))
            for li in (1, 2, 3):
                H, P, O, _ = LAYERS[li]
                load_small('L%d_bout' % li, (O, 1))
                load_small('L%d_skipb' % li, (O, 1))
                sk = singles.tile([H, O], F16, tag='L%d_skip' % li)
                nc.sync.dma_start(out=sk, in_=dr['L%d_skip' % li])
                smalls['L%d_skip' % li] = sk
                for comp in range(2):
                    for g in range(G):
                        t = singles.tile([128, T], F32, tag=f'cv{li}_{comp}_{g}')
                        nc.sync.dma_start(out=t, in_=dr['L%d_cv' % li][comp, g])
                        smalls[f'cv{li}_{comp}_{g}'] = t

            # r / r32 vectors (patterns regenerated per layer/chunk below)
            rvecs = {}
            for li in (1, 2, 3):
                for g in range(G):
                    rv = singles.tile([128, 1], F32, tag=f'r{li}_{g}',
                                      name=f'r{li}_{g}')
                    nc.sync.dma_start(out=rv, in_=dr['L%d_r' % li][g])
                    rvecs[(li, g)] = rv
            for li in range(4):
                for g in range(G):
                    rv = singles.tile([128, 1], F32, tag=f'r32v{li}_{g}')
                    nc.sync.dma_start(out=rv, in_=dr['L%d_r32' % li][g])
                    m = singles.tile([128, NBc], F32, tag=f'r32m{li}_{g}')
                    nc.vector.memset(m, 1.0)
                    nc.vector.tensor_scalar_mul(m, m, rv[:, 0:1])
                    r32m[(li, g)] = m

            # L0 b_in fold constants
            for c in range(2):
                for g in range(G):
                    bb = singles.tile([128, 1], F32, name=f'bbin_{c}_{g}',
                                      tag=f'bbin_{c}_{g}')
                    nc.sync.dma_start(out=bb, in_=dr['L0_bbin'][c, g])
                    smalls[f'bbin_{c}_{g}'] = bb

            # init Xtil carry cols to zero
            for k, t in xtil.items():
                nc.vector.memset(t[:, 0:1], 0.0)

            # complex multiply helper: out_re/out_im = (ar+i ai)(br+i bi)
            # a is a table (fp32 tiles), b given as APs (may be strided/fp16)
            def cmul(out_re, out_im, ar, ai, br, bi, tmp_shape):
                t1 = tmp_pool.tile(tmp_shape, F32, tag="cm1")
                t2 = tmp_pool.tile(tmp_shape, F32, tag="cm2")
                nc.vector.tensor_mul(t1, ar, br)
                nc.vector.tensor_mul(t2, ai, bi)
                nc.vector.tensor_sub(out_re, t1, t2)
                nc.vector.tensor_mul(t1, ar, bi)
                nc.vector.tensor_mul(t2, ai, br)
                nc.vector.tensor_add(out_im, t1, t2)

            # ---------------- main chunk loop ----------------
            for ch in range(n_chunks):
                bc0 = ch * NBc       # global block offset

                # ======== LAYER 0 ========
                H0, P0, O0, _ = LAYERS[0]
                bu_sb = {}
                xshift = {}
                for g in range(G):
                    bu_ps = {c: mm_ps.tile([128, NBc], F32, tag="mmps", name="bu_ps")
                             for c in range(2)}
                    for c in range(2):
                        nc.tensor.matmul(
                            bu_ps[c],
                            l0b_sb[:, c * 256 + g * 128: c * 256 + (g + 1) * 128],
                            midiT_sb[:, bc0:bc0 + NBc], start=True, stop=True)
                    # copies to fp16 (C2S rhs) with b_in fold
                    for c in range(2):
                        t = tmp_pool.tile([128, NBc], F16, tag=f"bu16_{g}_{c}")
                        nc.scalar.activation(out=t, in_=bu_ps[c],
                                             func=AF.Identity,
                                             bias=smalls[f'bbin_{c}_{g}'][:, 0:1],
                                             scale=1.0)
                        bu_sb[(g, c)] = t
                    # F' = t0pre o (Bu + bbin) ; use fp16 bu_sb (has bias)
                    tpre = {c: tab_pool.tile([128, NBc], F32, tag="tab", name="tpre")
                            for c in range(2)}
                    for c in range(2):
                        nc.sync.dma_start(out=tpre[c],
                                          in_=dr['L0_t0pre'][c, g, :, bc0:bc0 + NBc])
                    fp_re = tmp_pool.tile([128, NBc], F32, tag="fp_re")
                    fp_im = tmp_pool.tile([128, NBc], F32, tag="fp_im")
                    cmul(fp_re, fp_im, tpre[0], tpre[1],
                         bu_sb[(g, 0)], bu_sb[(g, 1)], [128, NBc])
                    # block scans
                    xb = {c: xtil[(0, c, g)] for c in range(2)}
                    for c, fp in ((0, fp_re), (1, fp_im)):
                        nc.vector.tensor_tensor_scan(
                            out=xb[c][:, 1:NBc + 1], data0=r32m[(0, g)],
                            data1=fp, initial=xb[c][:, 0:1],
                            op0=OP.mult, op1=OP.add)
                    # Xshift = t0post o Xtil[:, 0:NBc]  (fp16 out)
                    tpost = {c: tab_pool.tile([128, NBc], F32, tag="tab", name="tpost")
                             for c in range(2)}
                    for c in range(2):
                        nc.sync.dma_start(out=tpost[c],
                                          in_=dr['L0_t0post'][c, g, :, bc0:bc0 + NBc])
                    xs_re = tmp_pool.tile([128, NBc], F16, tag=f"xs_re{g}")
                    xs_im = tmp_pool.tile([128, NBc], F16, tag=f"xs_im{g}")
                    cmul(xs_re, xs_im, tpost[0], tpost[1],
                         xb[0][:, 0:NBc], xb[1][:, 0:NBc], [128, NBc])
                    xshift[(g, 0)], xshift[(g, 1)] = xs_re, xs_im
                    # carry col for next chunk
                    for c in range(2):
                        nc.vector.tensor_copy(xb[c][:, 0:1], xb[c][:, NBc:NBc + 1])

                # stripes: y = C1 @ Xshift + C2S @ Bu
                u_cur = u_a
                for j in range(T):
                    c1 = wc_pool.tile([128, 4 * O0], F16, tag="wc")
                    c2s = wc_pool.tile([128, 4 * O0], F16, tag="wc")
                    nc.sync.dma_start(out=c1, in_=dr['L0_c1'][j])
                    nc.sync.dma_start(out=c2s, in_=dr['L0_c2s'][j])
                    yp = y_ps_pool.tile([O0, NBc], F32, tag="yps")
                    first = True
                    for g in range(G):
                        for c in range(2):
                            nc.tensor.matmul(
                                yp, c1[:, (g * 2 + c) * O0:(g * 2 + c + 1) * O0],
                                xshift[(g, c)], start=first, stop=False)
                            first = False
                            nc.tensor.matmul(
                                yp, c2s[:, (g * 2 + c) * O0:(g * 2 + c + 1) * O0],
                                bu_sb[(g, c)],
                                start=False, stop=(g == G - 1 and c == 1))
                    nc.scalar.activation(
                        out=u_cur[0:O0, j * NBc:(j + 1) * NBc], in_=yp,
                        func=AF.Relu, bias=smalls['L0_bout'][:, 0:1], scale=1.0)

                # ======== LAYERS 1-3 ========
                for li in (1, 2, 3):
                    H, P, O, _ = LAYERS[li]
                    u_prev = u_cur
                    u_next = u_b if u_cur is u_a else u_a
                    W_sb = {}
                    for g in range(G):
                        # scan-reset pattern for this (layer, group)
                        p = pat_pool.tile([128, SEG], F32, tag="pat", name="pat")
                        nc.vector.memset(p, 1.0)
                        nc.vector.tensor_scalar_mul(p, p, rvecs[(li, g)][:, 0:1])
                        nc.vector.memset(p[:, ::T], 0.0)
                        pat[(li, g)] = p
                        # ---- stages A+C per comp through shared v ----
                        for c in range(2):
                            for j in range(T):
                                wb = wb_pool.tile([H, 512], F16, tag="wb")
                                nc.sync.dma_start(out=wb,
                                                  in_=dr['L%d_wb' % li][j])
                                ustripe = u_prev[0:H, j * NBc:(j + 1) * NBc]
                                vp = mm_ps.tile([128, NBc], F32, tag="mmps")
                                nc.tensor.matmul(
                                    vp, wb[:, c * 256 + g * 128:c * 256 + (g + 1) * 128],
                                    ustripe, start=True, stop=True)
                                nc.scalar.activation(
                                    out=v_buf[:, j::T], in_=vp, func=AF.Identity,
                                    bias=smalls[f'cv{li}_{c}_{g}'][:, j:j + 1],
                                    scale=1.0)
                            for s in range(NSEG):
                                nc.vector.tensor_tensor_scan(
                                    out=Z[(g, c)][:, s * SEG:(s + 1) * SEG],
                                    data0=pat[(li, g)],
                                    data1=v_buf[:, s * SEG:(s + 1) * SEG],
                                    initial=0.0, op0=OP.mult, op1=OP.add)
                        # ---- stage B: block pipeline ----
                        tf = {c: tab_pool.tile([128, NBc], F32, tag="tab", name="tf")
                              for c in range(2)}
                        tw = {c: tab_pool.tile([128, NBc], F32, tag="tab", name="tw")
                              for c in range(2)}
                        for c in range(2):
                            nc.sync.dma_start(out=tf[c],
                                              in_=dr['L%d_tabF' % li][c, g, :, bc0:bc0 + NBc])
                            nc.sync.dma_start(out=tw[c],
                                              in_=dr['L%d_tabW' % li][c, g, :, bc0:bc0 + NBc])
                        fp_re = tmp_pool.tile([128, NBc], F32, tag="fp_re")
                        fp_im = tmp_pool.tile([128, NBc], F32, tag="fp_im")
                        cmul(fp_re, fp_im, tf[0], tf[1],
                             Z[(g, 0)][:, T - 1::T], Z[(g, 1)][:, T - 1::T],
                             [128, NBc])
                        xb = {c: xtil[(li, c, g)] for c in range(2)}
                        for c, fp in ((0, fp_re), (1, fp_im)):
                            nc.vector.tensor_tensor_scan(
                                out=xb[c][:, 1:NBc + 1], data0=r32m[(li, g)],
                                data1=fp, initial=xb[c][:, 0:1],
                                op0=OP.mult, op1=OP.add)
                        w_re = tmp_pool.tile([128, NBc], F16, tag=f"w_re{g}")
                        w_im = tmp_pool.tile([128, NBc], F16, tag=f"w_im{g}")
                        cmul(w_re, w_im, tw[0], tw[1],
                             xb[0][:, 0:NBc], xb[1][:, 0:NBc], [128, NBc])
                        W_sb[(g, 0)], W_sb[(g, 1)] = w_re, w_im
                        for c in range(2):
                            nc.vector.tensor_copy(xb[c][:, 0:1], xb[c][:, NBc:NBc + 1])

                    # ---- stage D/E: y stripes ----
                    for j in range(T):
                        wc2 = wc_pool.tile([128, 4 * O], F16, tag="wc")
                        wc3 = wc_pool.tile([128, 4 * O], F16, tag="wc")
                        nc.sync.dma_start(out=wc2, in_=dr['L%d_wc2' % li][j])
                        nc.sync.dma_start(out=wc3, in_=dr['L%d_wc3' % li][j])
                        yp = y_ps_pool.tile([O, NBc], F32, tag="yps")
                        first = True
                        for g in range(G):
                            for c in range(2):
                                nc.tensor.matmul(
                                    yp, wc2[:, (g * 2 + c) * O:(g * 2 + c + 1) * O],
                                    Z[(g, c)][:, j::T], start=first, stop=False)
                                first = False
                                nc.tensor.matmul(
                                    yp, wc3[:, (g * 2 + c) * O:(g * 2 + c + 1) * O],
                                    W_sb[(g, c)],
                                    start=False, stop=(g == G - 1 and c == 1))
                        sp = s_ps_pool.tile([O, NBc], F32, tag="sps")
                        nc.tensor.matmul(sp, smalls['L%d_skip' % li],
                                         u_prev[0:H, j * NBc:(j + 1) * NBc],
                                         start=True, stop=True)
                        relu_t = tmp_pool.tile([O, NBc], F32, tag="relu")
                        nc.scalar.activation(out=relu_t, in_=yp, func=AF.Relu,
                                             bias=smalls['L%d_bout' % li][:, 0:1],
                                             scale=1.0)
                        # u_next = relu_t + skip + skip_b
                        add_t = tmp_pool.tile([O, NBc], F32, tag="addt")
                        nc.vector.tensor_add(add_t, relu_t, sp)
                        nc.vector.tensor_scalar_add(
                            u_next[0:O, j * NBc:(j + 1) * NBc], add_t,
                            smalls['L%d_skipb' % li][:, 0:1])
                    u_cur = u_next

                # ======== HEAD ========
                for j in range(T):
                    hp = y_ps_pool.tile([1, NBc], F32, tag="yps")
                    nc.tensor.matmul(hp, smalls['headW'],
                                     u_cur[0:20, j * NBc:(j + 1) * NBc],
                                     start=True, stop=True)
                    hstage = tmp_pool.tile([1, NBc], F32, tag="hstage")
                    nc.scalar.activation(
                        out=hstage, in_=hp, func=AF.Identity,
                        bias=smalls['headb'][:, 0:1], scale=1.0)
                    nc.sync.dma_start(out=audio_sb[j:j + 1, bc0:bc0 + NBc],
                                      in_=hstage)

            audio_scatter = bass.AP(tensor=audio_out.tensor, offset=0,
                                    ap=[[1, T], [T, NBLK]])
            nc.sync.dma_start(out=audio_scatter, in_=audio_sb)

    nc.compile()
    return nc


# ---------------- public entry -------------------------------------------
_NC_CACHE = {}
TRACE = False          # set True (e.g. from test.py) to collect an NTFF profile
LAST_RESULTS = None    # BassKernelResults of the most recent run


def kernel(midi, params):
    global LAST_RESULTS
    midi = np.asarray(midi)
    B, MIDI, H = midi.shape
    L = MIDI * T
    ins, midiT = preprocess(midi, params, L)
    key = L
    if key not in _NC_CACHE:
        _NC_CACHE[key] = build_nc(L)
    nc = _NC_CACHE[key]
    from concourse import bass_utils
    in_maps = [dict(ins, midiT=midiT[bi]) for bi in range(B)]
    res = bass_utils.run_bass_kernel_spmd(nc, in_maps, core_ids=list(range(B)),
                                          trace=TRACE)
    LAST_RESULTS = res
    audio = np.stack([np.asarray(res.results[bi]['audio']).reshape(L)
                      for bi in range(B)])
    return audio.reshape(B, L, 1).astype(np.float32)


# ---------------- self-test (CoreSim, small L) ----------------------------
def _ref_numpy(midi, params):
    """Literal float64 reimplementation of reference.py (no jax)."""
    B, MIDI, H = midi.shape
    x = np.repeat(np.asarray(midi, np.float64), T, axis=1)
    for li, (Hi, P, O, skip) in enumerate(LAYERS):
        lp = params['L%d' % li]
        lam, r, th, B_bar, C = _prep_layer(lp['ssm'])
        u = x + np.asarray(lp['ssm']['b_in'], np.float64)
        Bu = np.einsum('blh,ph->blp', u, B_bar)
        lam_bar = lam
        xs = np.zeros_like(Bu)
        s = np.zeros((B, P), complex)
        for t in range(x.shape[1]):
            s = lam_bar[None, :] * s + Bu[:, t]
            xs[:, t] = s
        y = 2.0 * np.einsum('blp,op->blo', xs, C).real + \
            np.asarray(lp['ssm']['b_out'], np.float64)
        y = np.maximum(y, 0.0) / SQRT2
        if skip:
            y = y + x @ np.asarray(lp['skip_W'], np.float64) + \
                np.asarray(lp['skip_b'], np.float64)
        x = y
    return x @ np.asarray(params['head_W'], np.float64) + \
        np.asarray(params['head_b'], np.float64)


def _make_params(rng):
    params = {}
    for i, (H, P, O, skip) in enumerate(LAYERS):
        ssm = dict(
            lam_re=rng.uniform(-1.0, -0.1, P).astype(np.float32),
            lam_im=rng.uniform(0.0, 3.14, P).astype(np.float32),
            B_re=(rng.randn(P, H) / np.sqrt(H)).astype(np.float32),
            B_im=(rng.randn(P, H) / np.sqrt(H)).astype(np.float32),
            C_re=(rng.randn(O, P) / np.sqrt(P)).astype(np.float32),
            C_im=(rng.randn(O, P) / np.sqrt(P)).astype(np.float32),
            b_in=np.zeros(H, np.float32),
            b_out=np.zeros(O, np.float32),
            log_step=rng.uniform(np.log(1e-3), np.log(1e-1), P).astype(np.float32),
        )
        lp = {'ssm': ssm}
        if skip:
            lp['skip_W'] = (rng.randn(H, O) / np.sqrt(H)).astype(np.float32)
            lp['skip_b'] = np.zeros(O, np.float32)
        params['L%d' % i] = lp
    params['head_W'] = (rng.randn(20, 1) / np.sqrt(20.0)).astype(np.float32)
    params['head_b'] = np.zeros(1, np.float32)
    return params


def _selftest(L=2048, sim=True):
    from concourse.bass_interp import CoreSim
    rng = np.random.RandomState(0)
    MIDI = L // T
    midi = rng.randn(1, MIDI, 88).astype(np.float32)
    params = _make_params(rng)
    expected = _ref_numpy(midi, params)          # [1, L, 1]

    ins, midiT = preprocess(midi, params, L)
    nc = build_nc(L)
    csim = CoreSim(nc, trace=False)
    for name, arr in dict(ins, midiT=midiT[0]).items():
        csim.tensor(name)[:] = arr
    csim.simulate(check_with_hw=False)
    actual = np.asarray(csim.tensor('audio')).reshape(1, L, 1)
    rel = np.linalg.norm(actual - expected) / np.linalg.norm(expected)
    print("selftest rel err:", rel)
    assert rel < 5e-3, f"selftest failed rel={rel}"
    print("SELFTEST OK")


if __name__ == "__main__":
    import sys
    _selftest(int(sys.argv[1]) if len(sys.argv) > 1 else 2048)


# revision 13
# speedup vs baseline: 1.3169x; 1.3169x over previous
"""PianoSSM Trainium2 kernel (nn_PianoSSM_XL).

Model: 4 stacked diagonal-complex SSM layers + linear head over
L = 512*32 = 16384 timesteps, batch 8. Data-parallel: one batch element
per NeuronCore (8 cores).

Math decomposition (per layer, lam_bar = r * e^{i theta} per channel):
  true state X[t] = sum_{s<=t} lam^{t-s} Bu[s]
  split t = 32*b + j.  Within a block, work in the rotated frame
  Z[b,j] = sum_{s<=j} r^{j-s} (e^{-i th s} Bu[32b+s])   (block-LOCAL, resets)
  - e^{-i th j} folded into 32 B-side weight variants (Btil_j)
  - Z computed by hardware tensor_tensor_scan (2 real scans per channel
    group; data0 pattern = r with 0 at block starts -> per-block reset)
  - block-local finals F_b = e^{i th 31} Z[b,31]; carries
    X_b = lam^32 X_{b-1} + F_b via a block-rate scan (rotation trick)
  - y[32b+j] = Re(C2_j Z[b,j]) + Re(C3_j W_b) + skip + bias, where
    W_b = e^{i th} X_{b-1},  C2_j = sqrt2 * C o e^{i th j},
    C3_j = C2_j o r^{j+1}    (relu prescale 1/sqrt2 folded into C-side)
  Layer 0 input is block-constant (midi repeated 32x): no full-rate scan;
  y = Re(C1_j X_{b-1}) + Re(C2S_j Bu_mid[b]) with C1_j = sqrt2*C o lam^{j+1},
  C2S_j = sqrt2*C o S_{j+1}, S_k = (lam^k - 1)/(lam - 1).

Layouts: everything "transposed" (features on partitions, time on free dim).
u buffers are j-major per chunk (col = j*NBc + b) so every per-j matmul
reads/writes contiguous stripes. v / Z are t-order (col = 32*b + j) for the
scans; per-j access is stride-32.
"""
import numpy as np

# ---------------- model constants (hardcoded per problem spec) -------------
T = 32                 # block size == COPY
LAYERS = [(88, 256, 88, False), (88, 256, 60, True), (60, 256, 40, True),
          (40, 256, 20, True)]
SQRT2 = float(np.sqrt(2.0))
N_CORES = 8
P_FULL = 256           # state size (2 groups of 128)
G = 2                  # channel groups


# ---------------- host-side weight preprocessing (float64) -----------------
def _prep_layer(ssm):
    dt = np.exp(np.asarray(ssm['log_step'], np.float64))
    lam = np.asarray(ssm['lam_re'], np.float64) + 1j * np.asarray(ssm['lam_im'], np.float64)
    lam_bar = np.exp(lam * dt)
    r = np.abs(lam_bar)
    th = np.angle(lam_bar)
    B = np.asarray(ssm['B_re'], np.float64) + 1j * np.asarray(ssm['B_im'], np.float64)
    B_bar = ((lam_bar - 1.0) / lam)[:, None] * B            # [P,H]
    C = np.asarray(ssm['C_re'], np.float64) + 1j * np.asarray(ssm['C_im'], np.float64)
    return lam_bar, r, th, B_bar, C


def _pack_cfam(Cfam):
    """Cfam: [T, O, P] complex -> dram [T, 128, 4*O] fp16 with im NEGATED.
    col layout: (g*2 + comp)*O + o ; partition k = K-row within group."""
    Tv, O, P = Cfam.shape
    out = np.zeros((Tv, 128, 4 * O), np.float16)
    for g in range(G):
        blk = Cfam[:, :, g * 128:(g + 1) * 128]             # [T,O,128]
        out[:, :, (g * 2 + 0) * O:(g * 2 + 1) * O] = \
            blk.real.transpose(0, 2, 1).astype(np.float16)
        out[:, :, (g * 2 + 1) * O:(g * 2 + 2) * O] = \
            (-blk.imag).transpose(0, 2, 1).astype(np.float16)
    return out


def _pack_bfam(Bfam):
    """Bfam: [T, P, H] complex -> dram [T, H, 512] fp16 (lhsT per comp/group).
    col = comp*256 + g*128 + m."""
    Tv, P, H = Bfam.shape
    out = np.zeros((Tv, H, 512), np.float16)
    for comp, arr in enumerate((Bfam.real, Bfam.imag)):
        for g in range(G):
            out[:, :, comp * 256 + g * 128: comp * 256 + (g + 1) * 128] = \
                arr[:, g * 128:(g + 1) * 128, :].transpose(0, 2, 1).astype(np.float16)
    return out


def _tab_pack(tab):
    """tab [P_FULL, N] complex -> [2, 2, 128, N] f32 (comp, g)."""
    N = tab.shape[1]
    out = np.zeros((2, G, 128, N), np.float32)
    for g in range(G):
        out[0, g] = tab[g * 128:(g + 1) * 128, :].real.astype(np.float32)
        out[1, g] = tab[g * 128:(g + 1) * 128, :].imag.astype(np.float32)
    return out


def preprocess(midi, params, L):
    """Returns (shared device input dict, per-core midiT list)."""
    NBLK = L // T
    MIDI = NBLK                       # midi steps == blocks (COPY == T)
    j = np.arange(T)
    b = np.arange(NBLK)
    ins = {}

    for li, (H, P, O, skip) in enumerate(LAYERS):
        lp = params['L%d' % li]
        lam, r, th, B_bar, C = _prep_layer(lp['ssm'])
        b_in = np.asarray(lp['ssm']['b_in'], np.float64)
        b_out = np.asarray(lp['ssm']['b_out'], np.float64)
        thT = th * T
        rT = r ** T
        sC = (2.0 / SQRT2) * C        # sqrt2 * C  (relu prescale folded)

        if li == 0:
            # --- all-matmul layer ---
            denom = np.where(np.abs(lam - 1.0) < 1e-30, 1.0, lam - 1.0)
            S = (lam[None, :] ** (j + 1)[:, None] - 1.0) / denom[None, :]  # S_{j+1} [T,P]
            C1 = sC[None] * (lam[None, :] ** (j + 1)[:, None])[:, None, :]   # [T,O,P]
            C2S = sC[None] * S[:, None, :]
            ins['L0_c1'] = _pack_cfam(C1)
            ins['L0_c2s'] = _pack_cfam(C2S)
            # B lhsT [88, 512] (col = comp*256 + m)
            bl = np.zeros((H, 512), np.float16)
            bl[:, 0:256] = B_bar.real.T.astype(np.float16)
            bl[:, 256:512] = B_bar.imag.T.astype(np.float16)
            ins['L0_b'] = bl
            ST = S[T - 1, :]          # sum_{i<T} lam^i
            t0pre = np.exp(-1j * thT[None, :] * b[:, None]).T * ST[:, None]   # [P,NBLK]
            ins['L0_t0pre'] = _tab_pack(t0pre)
            # t0post col c (global) multiplies Xtil_{c-1}: e^{+i thT (c-1)}
            t0post = np.exp(1j * thT[:, None] * (b - 1)[None, :])             # [P,NBLK]
            ins['L0_t0post'] = _tab_pack(t0post)
            ins['L0_r32'] = _tab_pack(rT[:, None].astype(np.complex128))[0]   # [2,128,1]
            bbin = B_bar @ b_in                                               # [P] complex
            ins['L0_bbin'] = _tab_pack(bbin[:, None])                         # [2,2,128,1]
            ins['L0_bout'] = (b_out / SQRT2).astype(np.float32)[:, None]
        else:
            rot_in = np.exp(-1j * th[None, :] * j[:, None])                   # [T,P]
            Btil = rot_in[:, :, None] * B_bar[None]                           # [T,P,H]
            ins['L%d_wb' % li] = _pack_bfam(Btil)
            C2 = sC[None] * np.exp(1j * th[None, :] * j[:, None])[:, None, :]  # [T,O,P]
            C3 = C2 * (r[None, :] ** (j + 1)[:, None])[:, None, :]
            ins['L%d_wc2' % li] = _pack_cfam(C2)
            ins['L%d_wc3' % li] = _pack_cfam(C3)
            ins['L%d_skip' % li] = np.asarray(lp['skip_W'], np.float64).astype(np.float16)
            # tabF: F_b = e^{i th (T-1)} Z_end; pre-rot for block scan e^{-i thT b}
            tabF = np.exp(1j * th[:, None] * (T - 1) - 1j * thT[:, None] * b[None, :])
            ins['L%d_tabF' % li] = _tab_pack(tabF)
            # tabW col b multiplies Xtil_{b-1}: W_b = e^{i th (T b - (T-1))} Xtil_{b-1}
            tabW = np.exp(1j * th[:, None] * (T * b[None, :] - (T - 1)))
            ins['L%d_tabW' % li] = _tab_pack(tabW)
            ins['L%d_r32' % li] = _tab_pack(rT[:, None].astype(np.complex128))[0]
            ins['L%d_r' % li] = _tab_pack(r[:, None].astype(np.complex128))[0]
            # b_in fold: cv[comp, g, :, j] = (Btil_j @ b_in)
            cv = np.einsum('tph,h->tp', Btil, b_in)                           # [T,P]
            cvp = np.zeros((2, G, 128, T), np.float32)
            for g in range(G):
                cvp[0, g] = cv.T[g * 128:(g + 1) * 128, :].real.astype(np.float32)
                cvp[1, g] = cv.T[g * 128:(g + 1) * 128, :].imag.astype(np.float32)
            ins['L%d_cv' % li] = cvp
            ins['L%d_bout' % li] = (b_out / SQRT2 +
                                    np.asarray(lp['skip_b'], np.float64)
                                    ).astype(np.float32)[:, None]
            # NOTE: skip_b folded into bout bias (added before relu would be
            # wrong -> handled below: bias arg of relu only gets b_out/sqrt2;
            # skip_b added via skip PSUM? We fold skip_b into the DVE add via
            # bias-free trick: skip_b is added to bout AFTER relu is wrong.
            # -> corrected in kernel: relu bias = b_out/sqrt2 only; skip_b
            # separate [O,1] added with the skip term.
            ins['L%d_bout' % li] = (b_out / SQRT2).astype(np.float32)[:, None]
            ins['L%d_skipb' % li] = np.asarray(lp['skip_b'], np.float64
                                               ).astype(np.float32)[:, None]

    ins['headW'] = np.asarray(params['head_W'], np.float64).astype(np.float16)  # [20,1]
    ins['headb'] = np.asarray(params['head_b'], np.float64).astype(np.float32)[:, None]  # [1,1]

    midiT = [np.ascontiguousarray(np.asarray(midi[bi], np.float32).T
                                  ).astype(np.float16) for bi in range(midi.shape[0])]
    return ins, midiT


# ---------------- device kernel ------------------------------------------
def build_nc(L, n_chunks=2, use_skipb=False, use_cv=False):
    import concourse.bass as bass
    import concourse.bacc as bacc
    import concourse.tile as tile
    from concourse import mybir

    F32 = mybir.dt.float32
    F16 = mybir.dt.float16
    AF = mybir.ActivationFunctionType
    OP = mybir.AluOpType

    NBLK = L // T
    MIDI = NBLK
    CT = L // n_chunks            # timesteps per chunk
    NBc = CT // T                 # blocks per chunk
    SEG_BLKS = min(32, NBc)
    SEG = SEG_BLKS * T            # scan segment cols
    NSEG = CT // SEG

    nc = bacc.Bacc("TRN2", target_bir_lowering=False, debug=False)

    # ---- DRAM I/O ----
    dr = {}
    def din(name, shape, dt):
        dr[name] = nc.dram_tensor(name, list(shape), dt, kind="ExternalInput").ap()

    din('midiT', (88, MIDI), F16)
    din('L0_b', (88, 512), F16)
    din('L0_c1', (T, 128, 4 * 88), F16)
    din('L0_c2s', (T, 128, 4 * 88), F16)
    din('L0_t0pre', (2, G, 128, MIDI), F32)
    din('L0_t0post', (2, G, 128, MIDI), F32)
    din('L0_r32', (2, 128, 1), F32)
    din('L0_bbin', (2, G, 128, 1), F32)
    din('L0_bout', (88, 1), F32)
    for li in (1, 2, 3):
        H, P, O, _ = LAYERS[li]
        din('L%d_wb' % li, (T, H, 512), F16)
        din('L%d_wc2' % li, (T, 128, 4 * O), F16)
        din('L%d_wc3' % li, (T, 128, 4 * O), F16)
        din('L%d_skip' % li, (H, O), F16)
        din('L%d_tabF' % li, (2, G, 128, NBLK), F32)
        din('L%d_tabW' % li, (2, G, 128, NBLK), F32)
        din('L%d_r32' % li, (2, 128, 1), F32)
        din('L%d_r' % li, (2, 128, 1), F32)
        din('L%d_cv' % li, (2, G, 128, T), F32)
        din('L%d_bout' % li, (O, 1), F32)
        din('L%d_skipb' % li, (O, 1), F32)
    din('headW', (20, 1), F16)
    din('headb', (1, 1), F32)
    audio_out = nc.dram_tensor("audio", [1, L], F32, kind="ExternalOutput").ap()

    with tile.TileContext(nc) as tc:
        with tc.tile_pool(name="singles", bufs=1) as singles, \
             tc.tile_pool(name="wb_pool", bufs=8) as wb_pool, \
             tc.tile_pool(name="wc_pool", bufs=8) as wc_pool, \
             tc.tile_pool(name="tab_pool", bufs=6) as tab_pool, \
             tc.tile_pool(name="tmp_pool", bufs=2) as tmp_pool, \
             tc.tile_pool(name="pat_pool", bufs=2) as pat_pool, \
             tc.tile_pool(name="mm_ps", bufs=3, space="PSUM") as mm_ps, \
             tc.tile_pool(name="y_ps", bufs=3, space="PSUM") as y_ps_pool, \
             tc.tile_pool(name="s_ps", bufs=2, space="PSUM") as s_ps_pool:

            # ---------- persistent SBUF ----------
            # u ping-pong (j-major per chunk) fp16
            u_a = singles.tile([88, CT], F16, tag="u_a")
            u_b = singles.tile([88, CT], F16, tag="u_b")
            # v (t-order, scan input) fp32 — shared serially across groups
            v_re = singles.tile([128, CT], F16, tag="v_re")
            v_im = singles.tile([128, CT], F16, tag="v_im")
            # Z per group fp16
            Z = {(g, c): singles.tile([128, CT], F16, tag=f"Z_{g}_{c}", name=f"Z_{g}_{c}")
                 for g in range(G) for c in range(2)}
            # scan reset patterns per (layer 1..3, group) [128, SEG]
            pat = {}
            # block-scan data0 (r^T) per (layer, group): [128, NBc]
            r32m = {}
            # Xtil carry buffers per (layer, comp, g): [128, NBc+1]
            xtil = {(li, c, g): singles.tile([128, NBc + 1], F16,
                                             tag=f"xt_{li}_{c}_{g}",
                                             name=f"xt_{li}_{c}_{g}")
                    for li in range(4) for c in range(2) for g in range(G)}
            audio_sb = singles.tile([T, NBLK], F32, tag="audio")

            # small per-layer constants kept resident
            smalls = {}
            def load_small(name, shape, dt=F32):
                t = singles.tile(list(shape), dt, tag=name)
                nc.sync.dma_start(out=t, in_=dr[name])
                smalls[name] = t
                return t

            midiT_sb = singles.tile([88, MIDI], F16, tag="midiT")
            nc.sync.dma_start(out=midiT_sb, in_=dr['midiT'])
            l0b_sb = singles.tile([88, 512], F16, tag="l0b")
            nc.sync.dma_start(out=l0b_sb, in_=dr['L0_b'])
            load_small('L0_bout', (88, 1))
            load_small('headW', (20, 1), F16)
            load_small('headb', (1, 1))
            for li in (1, 2, 3):
                H, P, O, _ = LAYERS[li]
                load_small('L%d_bout' % li, (O, 1))
                load_small('L%d_skipb' % li, (O, 1))
                sk = singles.tile([H, O], F16, tag='L%d_skip' % li)
                nc.sync.dma_start(out=sk, in_=dr['L%d_skip' % li])
                smalls['L%d_skip' % li] = sk
                for comp in range(2):
                    for g in range(G):
                        t = singles.tile([128, T], F32, tag=f'cv{li}_{comp}_{g}')
                        nc.sync.dma_start(out=t, in_=dr['L%d_cv' % li][comp, g])
                        smalls[f'cv{li}_{comp}_{g}'] = t

            # r / r32 vectors (patterns regenerated per layer/chunk below)
            rvecs = {}
            for li in (1, 2, 3):
                for g in range(G):
                    rv = singles.tile([128, 1], F32, tag=f'r{li}_{g}',
                                      name=f'r{li}_{g}')
                    nc.sync.dma_start(out=rv, in_=dr['L%d_r' % li][g])
                    rvecs[(li, g)] = rv
            for li in range(4):
                for g in range(G):
                    rv = singles.tile([128, 1], F32, tag=f'r32v{li}_{g}')
                    nc.sync.dma_start(out=rv, in_=dr['L%d_r32' % li][g])
                    m = singles.tile([128, NBc], F32, tag=f'r32m{li}_{g}')
                    nc.vector.memset(m, 1.0)
                    nc.vector.tensor_scalar_mul(m, m, rv[:, 0:1])
                    r32m[(li, g)] = m

            # L0 b_in fold constants
            for c in range(2):
                for g in range(G):
                    bb = singles.tile([128, 1], F32, name=f'bbin_{c}_{g}',
                                      tag=f'bbin_{c}_{g}')
                    nc.sync.dma_start(out=bb, in_=dr['L0_bbin'][c, g])
                    smalls[f'bbin_{c}_{g}'] = bb

            # init Xtil carry cols to zero
            for k, t in xtil.items():
                nc.vector.memset(t[:, 0:1], 0.0)

            # complex multiply helper: out_re/out_im = (ar+i ai)(br+i bi)
            # a is a table (fp32 tiles), b given as APs (may be strided/fp16)
            def cmul(out_re, out_im, ar, ai, br, bi, tmp_shape):
                t1 = tmp_pool.tile(tmp_shape, F32, tag="cm1")
                t2 = tmp_pool.tile(tmp_shape, F32, tag="cm2")
                nc.vector.tensor_mul(t1, ar, br)
                nc.vector.tensor_mul(t2, ai, bi)
                nc.vector.tensor_sub(out_re, t1, t2)
                nc.vector.tensor_mul(t1, ar, bi)
                nc.vector.tensor_mul(t2, ai, br)
                nc.vector.tensor_add(out_im, t1, t2)

            # ---------------- main chunk loop ----------------
            for ch in range(n_chunks):
                bc0 = ch * NBc       # global block offset

                # ======== LAYER 0 ========
                H0, P0, O0, _ = LAYERS[0]
                bu_sb = {}
                xshift = {}
                for g in range(G):
                    bu_ps = {c: mm_ps.tile([128, NBc], F32, tag="mmps", name="bu_ps")
                             for c in range(2)}
                    for c in range(2):
                        nc.tensor.matmul(
                            bu_ps[c],
                            l0b_sb[:, c * 256 + g * 128: c * 256 + (g + 1) * 128],
                            midiT_sb[:, bc0:bc0 + NBc], start=True, stop=True)
                    # copies to fp16 (C2S rhs) with b_in fold
                    for c in range(2):
                        t = tmp_pool.tile([128, NBc], F16, tag=f"bu16_{g}_{c}")
                        nc.scalar.activation(out=t, in_=bu_ps[c],
                                             func=AF.Identity,
                                             bias=smalls[f'bbin_{c}_{g}'][:, 0:1],
                                             scale=1.0)
                        bu_sb[(g, c)] = t
                    # F' = t0pre o (Bu + bbin) ; use fp16 bu_sb (has bias)
                    tpre = {c: tab_pool.tile([128, NBc], F32, tag="tab", name="tpre")
                            for c in range(2)}
                    for c in range(2):
                        nc.sync.dma_start(out=tpre[c],
                                          in_=dr['L0_t0pre'][c, g, :, bc0:bc0 + NBc])
                    fp_re = tmp_pool.tile([128, NBc], F32, tag="fp_re")
                    fp_im = tmp_pool.tile([128, NBc], F32, tag="fp_im")
                    cmul(fp_re, fp_im, tpre[0], tpre[1],
                         bu_sb[(g, 0)], bu_sb[(g, 1)], [128, NBc])
                    # block scans
                    xb = {c: xtil[(0, c, g)] for c in range(2)}
                    for c, fp in ((0, fp_re), (1, fp_im)):
                        nc.vector.tensor_tensor_scan(
                            out=xb[c][:, 1:NBc + 1], data0=r32m[(0, g)],
                            data1=fp, initial=xb[c][:, 0:1],
                            op0=OP.mult, op1=OP.add)
                    # Xshift = t0post o Xtil[:, 0:NBc]  (fp16 out)
                    tpost = {c: tab_pool.tile([128, NBc], F32, tag="tab", name="tpost")
                             for c in range(2)}
                    for c in range(2):
                        nc.sync.dma_start(out=tpost[c],
                                          in_=dr['L0_t0post'][c, g, :, bc0:bc0 + NBc])
                    xs_re = tmp_pool.tile([128, NBc], F16, tag=f"xs_re{g}")
                    xs_im = tmp_pool.tile([128, NBc], F16, tag=f"xs_im{g}")
                    cmul(xs_re, xs_im, tpost[0], tpost[1],
                         xb[0][:, 0:NBc], xb[1][:, 0:NBc], [128, NBc])
                    xshift[(g, 0)], xshift[(g, 1)] = xs_re, xs_im
                    # carry col for next chunk
                    for c in range(2):
                        nc.vector.tensor_copy(xb[c][:, 0:1], xb[c][:, NBc:NBc + 1])

                # stripes: y = C1 @ Xshift + C2S @ Bu
                u_cur = u_a
                for j in range(T):
                    c1 = wc_pool.tile([128, 4 * O0], F16, tag="wc")
                    c2s = wc_pool.tile([128, 4 * O0], F16, tag="wc")
                    nc.sync.dma_start(out=c1, in_=dr['L0_c1'][j])
                    nc.sync.dma_start(out=c2s, in_=dr['L0_c2s'][j])
                    yp = y_ps_pool.tile([O0, NBc], F32, tag="yps")
                    first = True
                    for g in range(G):
                        for c in range(2):
                            nc.tensor.matmul(
                                yp, c1[:, (g * 2 + c) * O0:(g * 2 + c + 1) * O0],
                                xshift[(g, c)], start=first, stop=False)
                            first = False
                            nc.tensor.matmul(
                                yp, c2s[:, (g * 2 + c) * O0:(g * 2 + c + 1) * O0],
                                bu_sb[(g, c)],
                                start=False, stop=(g == G - 1 and c == 1))
                    nc.scalar.activation(
                        out=u_cur[0:O0, j * NBc:(j + 1) * NBc], in_=yp,
                        func=AF.Relu, bias=smalls['L0_bout'][:, 0:1], scale=1.0)

                # ======== LAYERS 1-3 ========
                for li in (1, 2, 3):
                    H, P, O, _ = LAYERS[li]
                    u_prev = u_cur
                    u_next = u_b if u_cur is u_a else u_a
                    W_sb = {}
                    for g in range(G):
                        # scan-reset pattern for this (layer, group)
                        p = pat_pool.tile([128, SEG], F32, tag="pat", name="pat")
                        nc.vector.memset(p, 1.0)
                        nc.vector.tensor_scalar_mul(p, p, rvecs[(li, g)][:, 0:1])
                        nc.vector.memset(p[:, ::T], 0.0)
                        pat[(li, g)] = p
                        # ---- stage A: paired j-stripes -> contiguous-ish copy ----
                        for c, vdst in ((0, v_re), (1, v_im)):
                            for j0 in range(0, T, 2):
                                vp = mm_ps.tile([128, 2 * NBc], F32, tag="mmps",
                                                name="vp")
                                for k in range(2):
                                    j = j0 + k
                                    wb = wb_pool.tile([H, 512], F16, tag="wb")
                                    nc.sync.dma_start(out=wb,
                                                      in_=dr['L%d_wb' % li][j])
                                    nc.tensor.matmul(
                                        vp[:, k * NBc:(k + 1) * NBc],
                                        wb[:, c * 256 + g * 128:c * 256 + (g + 1) * 128],
                                        u_prev[0:H, j * NBc:(j + 1) * NBc],
                                        start=True, stop=True)
                                if use_cv:
                                    for k in range(2):
                                        j = j0 + k
                                        nc.scalar.activation(
                                            out=vdst[:, j::T],
                                            in_=vp[:, k * NBc:(k + 1) * NBc],
                                            func=AF.Identity,
                                            bias=smalls[f'cv{li}_{c}_{g}'][:, j:j + 1],
                                            scale=1.0)
                                else:
                                    # out cols {T*b + j0 + k}: dims (b, k)
                                    out_ap = bass.AP(
                                        tensor=vdst.tensor,
                                        offset=vdst.offset + j0,
                                        ap=[vdst.ap[0], [T, NBc], [1, 2]])
                                    in_ap = bass.AP(
                                        tensor=vp.tensor, offset=vp.offset,
                                        ap=[vp.ap[0], [1, NBc], [NBc, 2]])
                                    nc.scalar.activation(out=out_ap, in_=in_ap,
                                                         func=AF.Copy, scale=1.0)
                            for s in range(NSEG):
                                nc.vector.tensor_tensor_scan(
                                    out=Z[(g, c)][:, s * SEG:(s + 1) * SEG],
                                    data0=pat[(li, g)],
                                    data1=vdst[:, s * SEG:(s + 1) * SEG],
                                    initial=0.0, op0=OP.mult, op1=OP.add)
                        # ---- stage B: block pipeline ----
                        tf = {c: tab_pool.tile([128, NBc], F32, tag="tab", name="tf")
                              for c in range(2)}
                        tw = {c: tab_pool.tile([128, NBc], F32, tag="tab", name="tw")
                              for c in range(2)}
                        for c in range(2):
                            nc.sync.dma_start(out=tf[c],
                                              in_=dr['L%d_tabF' % li][c, g, :, bc0:bc0 + NBc])
                            nc.sync.dma_start(out=tw[c],
                                              in_=dr['L%d_tabW' % li][c, g, :, bc0:bc0 + NBc])
                        fp_re = tmp_pool.tile([128, NBc], F32, tag="fp_re")
                        fp_im = tmp_pool.tile([128, NBc], F32, tag="fp_im")
                        cmul(fp_re, fp_im, tf[0], tf[1],
                             Z[(g, 0)][:, T - 1::T], Z[(g, 1)][:, T - 1::T],
                             [128, NBc])
                        xb = {c: xtil[(li, c, g)] for c in range(2)}
                        for c, fp in ((0, fp_re), (1, fp_im)):
                            nc.vector.tensor_tensor_scan(
                                out=xb[c][:, 1:NBc + 1], data0=r32m[(li, g)],
                                data1=fp, initial=xb[c][:, 0:1],
                                op0=OP.mult, op1=OP.add)
                        w_re = tmp_pool.tile([128, NBc], F16, tag=f"w_re{g}")
                        w_im = tmp_pool.tile([128, NBc], F16, tag=f"w_im{g}")
                        cmul(w_re, w_im, tw[0], tw[1],
                             xb[0][:, 0:NBc], xb[1][:, 0:NBc], [128, NBc])
                        W_sb[(g, 0)], W_sb[(g, 1)] = w_re, w_im
                        for c in range(2):
                            nc.vector.tensor_copy(xb[c][:, 0:1], xb[c][:, NBc:NBc + 1])

                    # ---- stage D/E: y stripes ----
                    for j in range(T):
                        wc2 = wc_pool.tile([128, 4 * O], F16, tag="wc")
                        wc3 = wc_pool.tile([128, 4 * O], F16, tag="wc")
                        nc.sync.dma_start(out=wc2, in_=dr['L%d_wc2' % li][j])
                        nc.sync.dma_start(out=wc3, in_=dr['L%d_wc3' % li][j])
                        yp = y_ps_pool.tile([O, NBc], F32, tag="yps")
                        first = True
                        for g in range(G):
                            for c in range(2):
                                nc.tensor.matmul(
                                    yp, wc2[:, (g * 2 + c) * O:(g * 2 + c + 1) * O],
                                    Z[(g, c)][:, j::T], start=first, stop=False)
                                first = False
                                nc.tensor.matmul(
                                    yp, wc3[:, (g * 2 + c) * O:(g * 2 + c + 1) * O],
                                    W_sb[(g, c)],
                                    start=False, stop=(g == G - 1 and c == 1))
                        sp = s_ps_pool.tile([O, NBc], F32, tag="sps")
                        nc.tensor.matmul(sp, smalls['L%d_skip' % li],
                                         u_prev[0:H, j * NBc:(j + 1) * NBc],
                                         start=True, stop=True)
                        relu_t = tmp_pool.tile([O, NBc], F32, tag="relu")
                        nc.scalar.activation(out=relu_t, in_=yp, func=AF.Relu,
                                             bias=smalls['L%d_bout' % li][:, 0:1],
                                             scale=1.0)
                        # u_next = relu_t + skip  (skip_b folded into the skip
                        # matmul via an extra ones-column would cost more; the
                        # model's skip_b is zero -- asserted host-side)
                        if use_skipb:
                            add_t = tmp_pool.tile([O, NBc], F32, tag="addt")
                            nc.vector.tensor_add(add_t, relu_t, sp)
                            nc.vector.tensor_scalar_add(
                                u_next[0:O, j * NBc:(j + 1) * NBc], add_t,
                                smalls['L%d_skipb' % li][:, 0:1])
                        else:
                            nc.vector.tensor_add(
                                u_next[0:O, j * NBc:(j + 1) * NBc], relu_t, sp)
                    u_cur = u_next

                # ======== HEAD ========
                for j in range(T):
                    hp = y_ps_pool.tile([1, NBc], F32, tag="yps")
                    nc.tensor.matmul(hp, smalls['headW'],
                                     u_cur[0:20, j * NBc:(j + 1) * NBc],
                                     start=True, stop=True)
                    hstage = tmp_pool.tile([1, NBc], F32, tag="hstage")
                    nc.scalar.activation(
                        out=hstage, in_=hp, func=AF.Identity,
                        bias=smalls['headb'][:, 0:1], scale=1.0)
                    nc.sync.dma_start(out=audio_sb[j:j + 1, bc0:bc0 + NBc],
                                      in_=hstage)

            audio_scatter = bass.AP(tensor=audio_out.tensor, offset=0,
                                    ap=[[1, T], [T, NBLK]])
            nc.sync.dma_start(out=audio_scatter, in_=audio_sb)

    nc.compile()
    return nc


# ---------------- public entry -------------------------------------------
_NC_CACHE = {}
TRACE = False          # set True (e.g. from test.py) to collect an NTFF profile
LAST_RESULTS = None    # BassKernelResults of the most recent run


def kernel(midi, params):
    global LAST_RESULTS
    midi = np.asarray(midi)
    B, MIDI, H = midi.shape
    L = MIDI * T
    ins, midiT = preprocess(midi, params, L)
    use_skipb = any(np.any(np.asarray(params['L%d' % i]['skip_b']) != 0)
                    for i in (1, 2, 3))
    use_cv = any(np.any(np.asarray(params['L%d' % i]['ssm']['b_in']) != 0)
                 for i in range(4))
    key = (L, use_skipb, use_cv)
    if key not in _NC_CACHE:
        _NC_CACHE[key] = build_nc(L, use_skipb=use_skipb, use_cv=use_cv)
    nc = _NC_CACHE[key]
    from concourse import bass_utils
    in_maps = [dict(ins, midiT=midiT[bi]) for bi in range(B)]
    res = bass_utils.run_bass_kernel_spmd(nc, in_maps, core_ids=list(range(B)),
                                          trace=TRACE)
    LAST_RESULTS = res
    audio = np.stack([np.asarray(res.results[bi]['audio']).reshape(L)
                      for bi in range(B)])
    return audio.reshape(B, L, 1).astype(np.float32)


# ---------------- self-test (CoreSim, small L) ----------------------------
def _ref_numpy(midi, params):
    """Literal float64 reimplementation of reference.py (no jax)."""
    B, MIDI, H = midi.shape
    x = np.repeat(np.asarray(midi, np.float64), T, axis=1)
    for li, (Hi, P, O, skip) in enumerate(LAYERS):
        lp = params['L%d' % li]
        lam, r, th, B_bar, C = _prep_layer(lp['ssm'])
        u = x + np.asarray(lp['ssm']['b_in'], np.float64)
        Bu = np.einsum('blh,ph->blp', u, B_bar)
        lam_bar = lam
        xs = np.zeros_like(Bu)
        s = np.zeros((B, P), complex)
        for t in range(x.shape[1]):
            s = lam_bar[None, :] * s + Bu[:, t]
            xs[:, t] = s
        y = 2.0 * np.einsum('blp,op->blo', xs, C).real + \
            np.asarray(lp['ssm']['b_out'], np.float64)
        y = np.maximum(y, 0.0) / SQRT2
        if skip:
            y = y + x @ np.asarray(lp['skip_W'], np.float64) + \
                np.asarray(lp['skip_b'], np.float64)
        x = y
    return x @ np.asarray(params['head_W'], np.float64) + \
        np.asarray(params['head_b'], np.float64)


def _make_params(rng):
    params = {}
    for i, (H, P, O, skip) in enumerate(LAYERS):
        ssm = dict(
            lam_re=rng.uniform(-1.0, -0.1, P).astype(np.float32),
            lam_im=rng.uniform(0.0, 3.14, P).astype(np.float32),
            B_re=(rng.randn(P, H) / np.sqrt(H)).astype(np.float32),
            B_im=(rng.randn(P, H) / np.sqrt(H)).astype(np.float32),
            C_re=(rng.randn(O, P) / np.sqrt(P)).astype(np.float32),
            C_im=(rng.randn(O, P) / np.sqrt(P)).astype(np.float32),
            b_in=np.zeros(H, np.float32),
            b_out=np.zeros(O, np.float32),
            log_step=rng.uniform(np.log(1e-3), np.log(1e-1), P).astype(np.float32),
        )
        lp = {'ssm': ssm}
        if skip:
            lp['skip_W'] = (rng.randn(H, O) / np.sqrt(H)).astype(np.float32)
            lp['skip_b'] = np.zeros(O, np.float32)
        params['L%d' % i] = lp
    params['head_W'] = (rng.randn(20, 1) / np.sqrt(20.0)).astype(np.float32)
    params['head_b'] = np.zeros(1, np.float32)
    return params


def _selftest(L=2048, sim=True):
    from concourse.bass_interp import CoreSim
    rng = np.random.RandomState(0)
    MIDI = L // T
    midi = rng.randn(1, MIDI, 88).astype(np.float32)
    params = _make_params(rng)
    expected = _ref_numpy(midi, params)          # [1, L, 1]

    ins, midiT = preprocess(midi, params, L)
    nc = build_nc(L)
    csim = CoreSim(nc, trace=False)
    for name, arr in dict(ins, midiT=midiT[0]).items():
        csim.tensor(name)[:] = arr
    csim.simulate(check_with_hw=False)
    actual = np.asarray(csim.tensor('audio')).reshape(1, L, 1)
    rel = np.linalg.norm(actual - expected) / np.linalg.norm(expected)
    print("selftest rel err:", rel)
    assert rel < 5e-3, f"selftest failed rel={rel}"
    print("SELFTEST OK")


if __name__ == "__main__":
    import sys
    _selftest(int(sys.argv[1]) if len(sys.argv) > 1 else 2048)
